# revision 1
# baseline (speedup 1.0000x reference)
"""CrystalGraphConvNet (CGCNN) forward pass on 8 Trainium2 NeuronCores.

Strategy (data-parallel over atoms, feature-major on chip):
  - 20000 atoms sharded 2500/core, padded to 2560 (= 5 blocks of 512).
  - Activations live feature-major in SBUF: x[ot] = [128 chan, 2560 atoms].
  - Per conv layer:
      * Y' = x @ w1_nbr computed once per atom (atom-major out via
        stationary-x matmuls), cast bf16, AllGather -> Y'full [20480, 512].
      * Neighbor messages fetched with dma_gather(transpose=True) straight
        into feature-major bf16 tiles and injected into the PSUM
        pre-activation accumulation with identity matmuls.
      * pre = x@w1_self + nbr_fea@w1_edge + Y'[idx] accumulated in PSUM;
        activation (relu / softplus=Ln(Exp+1)) with b1 as per-partition bias;
        sum over the 12 neighbor slots via identity matmuls into PSUM.
      * t = nbr_sum @ w2 ; BN train-mode stats via bn_stats/bn_aggr over the
        2500 valid atom columns + one tiny AllReduce; b2 dropped (BN mean
        subtraction cancels it); x' = softplus(x + a*t + b).
  - Pooling: x3 transposed to atom-major (PE transpose), AllGather fp32,
    dma_gather of the M1 rows, segment-mean via matmul with a host-built
    selection matrix (1/count baked in), then the tiny output MLP.
Host side only reshapes/shards/remaps indices; all FLOPs are on device.
"""

import os
import numpy as np
import ml_dtypes

import concourse.bacc as bacc
import concourse.bass as bass
import concourse.mybir as mybir
import concourse.tile as tile
from concourse.bass_utils import run_bass_kernel_spmd
from concourse.masks import make_identity

F32 = mybir.dt.float32
BF16 = mybir.dt.bfloat16
I16 = mybir.dt.int16
AF = mybir.ActivationFunctionType


class CFG:
    def __init__(self, N=20000, M=12, NBR=64, AFD=256, ORIG=92, H=256, B=200,
                 K=50, N_CONV=3, NC=8, EPS=1e-5):
        self.N, self.M, self.NBR, self.AFD, self.ORIG, self.H = N, M, NBR, AFD, ORIG, H
        self.B, self.K, self.N_CONV, self.NC, self.EPS = B, K, N_CONV, NC, EPS
        assert N % NC == 0
        self.NV = N // NC                    # valid atoms per core
        self.NLP = -(-self.NV // 512) * 512  # padded per-core atoms
        self.ABLK = 512
        self.NAB = self.NLP // self.ABLK     # 512-col tiles per core
        self.NJB = self.NLP // 128           # 128-col blocks per core
        self.GN = NC * self.NLP              # global padded rows
        self.OT = AFD // 128                 # 2 out-feature tiles
        self.CT = 2 * AFD // 128             # 4 hidden tiles
        self.M1 = M // 2
        assert B % NC == 0
        self.SEGC = B // NC                  # segments per core
        self.BK = B * K // NC                # m1 rows per core
        self.BKP = -(-self.BK // 128) * 128  # padded
        self.NMB = self.BKP // 128
        # bn_stats chunking of the NV valid columns
        nch = 1
        while self.NV // nch > 512 or self.NV % nch:
            nch += 1
        self.BN_NCH, self.BN_W = nch, self.NV // nch


def wrap16(idx, pad_to):
    """int16 index layout for dma_gather: [128, pad_to//16]."""
    a = np.zeros(pad_to, np.int16)
    a[: len(idx)] = idx.astype(np.int16)
    return np.tile(a.reshape(-1, 16).T, (8, 1))


def build_program(cfg: CFG):
    c = cfg
    nc = bacc.Bacc("TRN2", target_bir_lowering=False, debug=False, num_devices=c.NC)
    D = {}

    def din(name, shape, dt=F32):
        D[name] = nc.dram_tensor(name, list(shape), dt, kind="ExternalInput")
        return D[name]

    # per-core inputs
    din("atomT", [c.ORIG, c.NLP])                       # embed rhs (zero-padded)
    din("nbrT", [c.M, c.NBR, c.NLP])                    # edge features, feature-major
    din("gidx", [128, c.M * c.NAB * (c.ABLK // 16)], I16)
    din("m1idx", [128, c.BKP // 16], I16)
    din("sel", [128, c.NMB * c.SEGC])                   # segment-mean selection
    din("m2T", [4, c.SEGC])
    # shared weights
    din("emb_w", [c.ORIG, c.AFD])
    din("emb_b", [128, c.OT])
    din("w1s", [c.N_CONV, 2, 128, 2 * c.AFD])           # lhsT k-tiles
    din("w1n", [c.N_CONV, 2, 128, 2 * c.AFD])           # rhs k-tiles
    din("w1e", [c.N_CONV, c.NBR, 2 * c.AFD])
    din("b1", [128, c.N_CONV * c.CT])
    din("w2", [c.N_CONV, c.CT, 128, c.AFD])
    din("gamma", [128, c.N_CONV * c.OT])
    din("beta", [128, c.N_CONV * c.OT])
    din("fc_w0", [128, c.H])
    din("fc_w1", [128, c.H])
    din("fc_w2", [4, c.H])
    din("fc_b", [128, c.H // 128])
    din("out_w", [128, c.H // 128])
    din("out_b", [1, 1])
    out = nc.dram_tensor("o_out", [c.SEGC], F32, kind="ExternalOutput")

    NV, NLP, ABLK, NAB, NJB, OT, CT, M = (
        c.NV, c.NLP, c.ABLK, c.NAB, c.NJB, c.OT, c.CT, c.M)
    H2 = 2 * c.AFD
    rg = [list(range(c.NC))]

    KP = int(os.environ.get("KP", "99"))
    with tile.TileContext(nc) as tc:
        with (
            tc.tile_pool(name="persist", bufs=1) as pp,
            tc.tile_pool(name="wts", bufs=2) as wp,
            tc.tile_pool(name="io", bufs=3) as iop,
            tc.tile_pool(name="gat", bufs=3) as gp,
            tc.tile_pool(name="act", bufs=4) as ap_,
            tc.tile_pool(name="gsbp", bufs=2) as gbp,
            tc.tile_pool(name="embp", bufs=1) as ep,
            tc.tile_pool(name="small", bufs=2) as sp,
            tc.tile_pool(name="ps", bufs=4, space="PSUM") as ps,
            tc.tile_pool(name="gsum", bufs=1, space="PSUM") as gs,
            tc.tile_pool(name="dram", bufs=2, space="DRAM") as dp,
        ):
            # ---- persistent state ----
            x = [pp.tile([128, NLP], F32, tag=f"x{o}", name=f"x{o}") for o in range(OT)]
            t = [pp.tile([128, NLP], F32, tag=f"t{o}", name=f"t{o}") for o in range(OT)]
            ident = pp.tile([128, 128], BF16, tag="ident", name="ident")
            make_identity(nc, ident[:])
            identf = pp.tile([128, 128], F32, tag="identf", name="identf")
            make_identity(nc, identf[:])
            gix = pp.tile([128, M * NAB * (ABLK // 16)], I16, tag="gix")
            nc.sync.dma_start(gix[:], D["gidx"][:])
            embb = pp.tile([128, OT], F32, tag="embb", name="embb")
            nc.sync.dma_start(embb[:], D["emb_b"][:])
            b1t = pp.tile([128, c.N_CONV * CT], F32, tag="b1t", name="b1t")
            nc.sync.dma_start(b1t[:], D["b1"][:])
            gmt = pp.tile([128, c.N_CONV * OT], F32, tag="gmt", name="gmt")
            nc.sync.dma_start(gmt[:], D["gamma"][:])
            bet = pp.tile([128, c.N_CONV * OT], F32, tag="bet", name="bet")
            nc.sync.dma_start(bet[:], D["beta"][:])

            # ---- embedding ----
            ew = ep.tile([c.ORIG, c.AFD], F32, tag="ew", name="ew")
            nc.sync.dma_start(ew[:], D["emb_w"][:])
            at = ep.tile([c.ORIG, NLP], F32, tag="at", name="at")
            nc.sync.dma_start(at[:], D["atomT"][:])
            for ab in range(NAB):
                sl = slice(ab * ABLK, (ab + 1) * ABLK)
                for o in range(OT):
                    pt = ps.tile([128, ABLK], F32, tag="ps", name="ps")
                    nc.tensor.matmul(pt[:], ew[:, o * 128:(o + 1) * 128],
                                     at[:, sl], start=True, stop=True)
                    e = ap_.tile([128, ABLK], BF16, tag="sp_e", name="sp_e")
                    nc.scalar.activation(e[:], pt[:], AF.Exp, bias=embb[:, o:o + 1])
                    nc.scalar.activation(x[o][:, sl], e[:], AF.Ln, bias=1.0)

            # ---- conv layers ----
            NCONV_RUN = 0 if KP < 2 else (1 if KP < 5 else c.N_CONV)
            for li in range(NCONV_RUN):
                w1s = wp.tile([128, 2 * H2], F32, tag="w1s", name="w1s")
                w1n = wp.tile([128, 2 * H2], F32, tag="w1n", name="w1n")
                for k in range(2):
                    nc.sync.dma_start(w1s[:, k * H2:(k + 1) * H2], D["w1s"][li, k])
                    nc.sync.dma_start(w1n[:, k * H2:(k + 1) * H2], D["w1n"][li, k])
                w1e = wp.tile([c.NBR, H2], F32, tag="w1e", name="w1e")
                nc.sync.dma_start(w1e[:], D["w1e"][li])
                w2 = wp.tile([128, CT * c.AFD], F32, tag="w2", name="w2")
                for k in range(CT):
                    nc.sync.dma_start(w2[:, k * c.AFD:(k + 1) * c.AFD], D["w2"][li, k])

                # Y' = x @ w1n  (atom-major, bf16) -> AllGather
                yloc = dp.tile([NLP, H2], BF16, tag="yloc", name="yloc")
                yfull = dp.tile([c.GN, H2], BF16, tag="yfull", name="yfull", addr_space="Shared")
                for jb in range(NJB):
                    js = slice(jb * 128, (jb + 1) * 128)
                    pt = ps.tile([128, H2], F32, tag="ps", name="ps")
                    for k in range(OT):
                        nc.tensor.matmul(pt[:], x[k][:, js],
                                         w1n[:, k * H2:(k + 1) * H2],
                                         start=(k == 0), stop=(k == OT - 1))
                    yb = ap_.tile([128, H2], BF16, tag="yb", name="yb")
                    nc.scalar.activation(yb[:], pt[:], AF.Copy)
                    nc.sync.dma_start(yloc[js, :], yb[:])
                nc.gpsimd.collective_compute(
                    "AllGather", mybir.AluOpType.bypass, replica_groups=rg,
                    ins=[yloc.opt()], outs=[yfull.opt()])

                # edge phase
                for ab in range(NAB if KP >= 3 else 0):
                    sl = slice(ab * ABLK, (ab + 1) * ABLK)
                    gsum = [gs.tile([128, ABLK], F32, tag=f"gs{ctt}", name=f"gs{ctt}") for ctt in range(CT)]
                    for m in range(M):
                        yg = gp.tile([128, CT, ABLK], BF16, tag="yg", name="yg")
                        iw = ABLK // 16
                        nc.gpsimd.dma_gather(
                            yg[:], yfull[:],
                            gix[:, (m * NAB + ab) * iw:(m * NAB + ab + 1) * iw],
                            ABLK, ABLK, H2, transpose=True)
                        nb = iop.tile([c.NBR, ABLK], F32, tag="nb", name="nb")
                        nc.sync.dma_start(nb[:], D["nbrT"][m, :, sl])
                        for ct in range(CT):
                            cs = slice(ct * 128, (ct + 1) * 128)
                            pre = ps.tile([128, ABLK], F32, tag="ps", name="ps")
                            for k in range(OT):
                                nc.tensor.matmul(
                                    pre[:], w1s[:, k * H2:(k + 1) * H2][:, cs],
                                    x[k][:, sl], start=(k == 0), stop=False)
                            nc.tensor.matmul(pre[:], w1e[:, cs], nb[:],
                                             start=False, stop=False)
                            nc.tensor.matmul(pre[:], ident[:], yg[:, ct, :],
                                             start=False, stop=True)
                            g = ap_.tile([128, ABLK], BF16, tag="g", name="g")
                            bias = b1t[:, li * CT + ct:li * CT + ct + 1]
                            if m < c.M1:
                                nc.scalar.activation(g[:], pre[:], AF.Relu, bias=bias)
                            else:
                                e = ap_.tile([128, ABLK], BF16, tag="sp_e", name="sp_e")
                                nc.scalar.activation(e[:], pre[:], AF.Exp, bias=bias)
                                nc.scalar.activation(g[:], e[:], AF.Ln, bias=1.0)
                            nc.tensor.matmul(gsum[ct][:], ident[:], g[:],
                                             start=(m == 0), stop=(m == M - 1))
                    # t = gsum @ w2
                    gsb = gbp.tile([128, CT * ABLK], F32, tag="gsb", name="gsb")
                    for ct in range(CT):
                        nc.vector.tensor_copy(
                            gsb[:, ct * ABLK:(ct + 1) * ABLK], gsum[ct][:])
                    for o in range(OT):
                        pt = ps.tile([128, ABLK], F32, tag="ps", name="ps")
                        for ct in range(CT):
                            nc.tensor.matmul(
                                pt[:], w2[:, ct * c.AFD:(ct + 1) * c.AFD][:, o * 128:(o + 1) * 128],
                                gsb[:, ct * ABLK:(ct + 1) * ABLK],
                                start=(ct == 0), stop=(ct == CT - 1))
                        nc.scalar.activation(t[o][:, sl], pt[:], AF.Copy)

                if KP < 4:
                    continue
                # ---- BN stats (valid cols only) + AllReduce ----
                stats = sp.tile([128, 2 * OT], F32, tag="stats", name="stats")
                mv = sp.tile([128, 2 * OT], F32, tag="mv", name="mv")
                st6 = sp.tile([128, c.BN_NCH * 6], F32, tag="st6", name="st6")
                for o in range(OT):
                    for ch in range(c.BN_NCH):
                        nc.vector.bn_stats(
                            st6[:, ch * 6:(ch + 1) * 6],
                            t[o][:, ch * c.BN_W:(ch + 1) * c.BN_W])
                    nc.vector.bn_aggr(mv[:, 2 * o:2 * o + 2], st6[:])
                    # S1 = mean*NV ; S2 = (var + mean^2)*NV
                    sq = sp.tile([128, 1], F32, tag="sq", name="sq")
                    nc.vector.tensor_tensor(sq[:], mv[:, 2 * o:2 * o + 1],
                                            mv[:, 2 * o:2 * o + 1], op=mybir.AluOpType.mult)
                    nc.vector.tensor_tensor(sq[:], sq[:], mv[:, 2 * o + 1:2 * o + 2],
                                            op=mybir.AluOpType.add)
                    nc.vector.tensor_scalar(stats[:, 2 * o + 1:2 * o + 2], sq[:],
                                            float(NV), None, op0=mybir.AluOpType.mult)
                    nc.vector.tensor_scalar(stats[:, 2 * o:2 * o + 1],
                                            mv[:, 2 * o:2 * o + 1],
                                            float(NV), None, op0=mybir.AluOpType.mult)
                bn_i = dp.tile([128, 2 * OT], F32, tag="bn_i", name="bn_i")
                bn_o = dp.tile([128, 2 * OT], F32, tag="bn_o", name="bn_o", addr_space="Shared")
                nc.gpsimd.dma_start(bn_i[:], stats[:])
                nc.gpsimd.collective_compute(
                    "AllReduce", mybir.AluOpType.add, replica_groups=rg,
                    ins=[bn_i.opt()], outs=[bn_o.opt()])
                sg = sp.tile([128, 2 * OT], F32, tag="sg", name="sg")
                nc.sync.dma_start(sg[:], bn_o[:])
                # ---- BN apply + residual + softplus ----
                for o in range(OT):
                    lot = li * OT + o
                    mu = sp.tile([128, 1], F32, tag="mu", name="mu")
                    va = sp.tile([128, 1], F32, tag="va", name="va")
                    avec = sp.tile([128, 1], F32, tag="avec", name="avec")
                    bvec = sp.tile([128, 1], F32, tag="bvec", name="bvec")
                    nc.vector.tensor_scalar(mu[:], sg[:, 2 * o:2 * o + 1],
                                            1.0 / c.N, None, op0=mybir.AluOpType.mult)
                    nc.vector.tensor_scalar(va[:], sg[:, 2 * o + 1:2 * o + 2],
                                            1.0 / c.N, None, op0=mybir.AluOpType.mult)
                    nc.vector.tensor_tensor(bvec[:], mu[:], mu[:], op=mybir.AluOpType.mult)
                    nc.vector.tensor_tensor(va[:], va[:], bvec[:], op=mybir.AluOpType.subtract)
                    # rsqrt(va+eps) = exp(-0.5*ln(va+eps))
                    nc.vector.tensor_scalar(va[:], va[:], float(c.EPS), None,
                                            op0=mybir.AluOpType.add)
                    nc.scalar.activation(avec[:], va[:], AF.Ln)
                    nc.scalar.activation(avec[:], avec[:], AF.Exp, scale=-0.5)
                    nc.vector.tensor_tensor(avec[:], avec[:], gmt[:, lot:lot + 1],
                                            op=mybir.AluOpType.mult)
                    nc.vector.tensor_tensor(bvec[:], mu[:], avec[:], op=mybir.AluOpType.mult)
                    nc.vector.tensor_tensor(bvec[:], bet[:, lot:lot + 1], bvec[:],
                                            op=mybir.AluOpType.subtract)
                    for ab in range(NAB):
                        sl = slice(ab * ABLK, (ab + 1) * ABLK)
                        u = ap_.tile([128, ABLK], F32, tag="u", name="u")
                        nc.vector.tensor_scalar(u[:], t[o][:, sl], avec[:, 0:1],
                                                bvec[:, 0:1], op0=mybir.AluOpType.mult,
                                                op1=mybir.AluOpType.add)
                        nc.vector.tensor_tensor(u[:], u[:], x[o][:, sl],
                                                op=mybir.AluOpType.add)
                        e = ap_.tile([128, ABLK], BF16, tag="sp_e", name="sp_e")
                        nc.scalar.activation(e[:], u[:], AF.Exp)
                        nc.scalar.activation(x[o][:, sl], e[:], AF.Ln, bias=1.0)

            # ---- pooling + head ----
            if KP < 6:
                dbg = sp.tile([1, c.SEGC], F32, tag="dbg", name="dbg")
                nc.vector.tensor_copy(dbg[:], x[0][0:1, 0:c.SEGC])
                nc.sync.dma_start(out[None, :], dbg[:])
            if KP >= 6:
                xloc = dp.tile([NLP, c.AFD], F32, tag="xloc", name="xloc")
                xfull = dp.tile([c.GN, c.AFD], F32, tag="xfull", name="xfull", addr_space="Shared")
                for jb in range(NJB):
                    js = slice(jb * 128, (jb + 1) * 128)
                    xa = ap_.tile([128, c.AFD], F32, tag="xa", name="xa")
                    for o in range(OT):
                        pt = ps.tile([128, ABLK], F32, tag="ps", name="ps")
                        nc.tensor.transpose(pt[:, 0:128], x[o][:, js], identf[:])
                        nc.vector.tensor_copy(xa[:, o * 128:(o + 1) * 128], pt[:, 0:128])
                    nc.sync.dma_start(xloc[js, :], xa[:])
                nc.gpsimd.collective_compute(
                    "AllGather", mybir.AluOpType.bypass, replica_groups=rg,
                    ins=[xloc.opt()], outs=[xfull.opt()])
                if KP >= 7:
                    m1x = pp.tile([128, c.BKP // 16], I16, tag="m1x", name="m1x")
                    nc.sync.dma_start(m1x[:], D["m1idx"][:])
                    m1g = pp.tile([128, c.NMB, c.AFD], F32, tag="m1g", name="m1g")
                    # dma_gather caps near 1024 idxs (64-desc/engine packet) - chunk
                    for g0 in range(0, c.BKP, 512):
                        gn = min(512, c.BKP - g0)
                        nc.gpsimd.dma_gather(
                            m1g[:, g0 // 128:(g0 + gn) // 128, :], xfull[:],
                            m1x[:, g0 // 16:(g0 + gn) // 16], gn, gn, c.AFD)
                    selt = pp.tile([128, c.NMB * c.SEGC], F32, tag="selt", name="selt")
                    nc.sync.dma_start(selt[:], D["sel"][:])
                    crys = [gs.tile([128, c.SEGC], F32, tag=f"gs{o}", name=f"gs{o}") for o in range(OT)]
                    for b in range(c.NMB):
                        for o in range(OT):
                            nc.tensor.matmul(
                                crys[o][:], m1g[:, b, o * 128:(o + 1) * 128],
                                selt[:, b * c.SEGC:(b + 1) * c.SEGC],
                                start=(b == 0), stop=(b == c.NMB - 1))
                if KP == 7:
                    dbg2 = sp.tile([1, c.SEGC], F32, tag="dbg", name="dbg2")
                    nc.vector.tensor_copy(dbg2[:], crys[0][0:1, :])
                    nc.sync.dma_start(out[None, :], dbg2[:])
                if KP >= 8:
                    # softplus(crys_cat)
                    spc = [ap_.tile([128, c.SEGC], F32, tag=f"spc{o}", name=f"spc{o}") for o in range(OT)]
                    for o in range(OT):
                        e = ap_.tile([128, c.SEGC], F32, tag="spe2", name="spe2")
                        nc.scalar.activation(e[:], crys[o][:], AF.Exp)
                        nc.scalar.activation(spc[o][:], e[:], AF.Ln, bias=1.0)
                    m2t = sp.tile([4, c.SEGC], F32, tag="m2t", name="m2t")
                    nc.sync.dma_start(m2t[:], D["m2T"][:])
                    spm = sp.tile([4, c.SEGC], F32, tag="spm", name="spm")
                    nc.scalar.activation(spm[:], m2t[:], AF.Exp)
                    nc.scalar.activation(spm[:], spm[:], AF.Ln, bias=1.0)
                    fw = [wp.tile([128, c.H], F32, tag=f"fw{k}", name=f"fw{k}") for k in range(2)]
                    nc.sync.dma_start(fw[0][:], D["fc_w0"][:])
                    nc.sync.dma_start(fw[1][:], D["fc_w1"][:])
                    fw2 = sp.tile([4, c.H], F32, tag="fw2", name="fw2")
                    nc.sync.dma_start(fw2[:], D["fc_w2"][:])
                    fbt = sp.tile([128, c.H // 128], F32, tag="fbt", name="fbt")
                    nc.sync.dma_start(fbt[:], D["fc_b"][:])
                    owt = sp.tile([128, c.H // 128], F32, tag="owt", name="owt")
                    nc.sync.dma_start(owt[:], D["out_w"][:])
                    obt = sp.tile([1, 1], F32, tag="obt", name="obt")
                    nc.sync.dma_start(obt[:], D["out_b"][:])
                    hh = [ap_.tile([128, c.SEGC], F32, tag=f"hh{o}", name=f"hh{o}") for o in range(c.H // 128)]
                    for o in range(c.H // 128):
                        cs = slice(o * 128, (o + 1) * 128)
                        pt = ps.tile([128, ABLK], F32, tag="ps", name="ps")
                        nc.tensor.matmul(pt[:, 0:c.SEGC], fw[0][:, cs], spc[0][:], start=True, stop=False)
                        nc.tensor.matmul(pt[:, 0:c.SEGC], fw[1][:, cs], spc[1][:], start=False, stop=False)
                        nc.tensor.matmul(pt[:, 0:c.SEGC], fw2[:, cs], spm[:], start=False, stop=True)
                        e = ap_.tile([128, c.SEGC], F32, tag="spe2", name="spe2")
                        nc.scalar.activation(e[:], pt[:, 0:c.SEGC], AF.Exp, bias=fbt[:, o:o + 1])
                        nc.scalar.activation(hh[o][:], e[:], AF.Ln, bias=1.0)
                    po = ps.tile([128, ABLK], F32, tag="ps", name="ps")
                    for o in range(c.H // 128):
                        nc.tensor.matmul(po[0:1, 0:c.SEGC], owt[:, o:o + 1], hh[o][:],
                                         start=(o == 0), stop=(o == c.H // 128 - 1))
                    ov = sp.tile([1, c.SEGC], F32, tag="ov", name="ov")
                    nc.scalar.activation(ov[:], po[0:1, 0:c.SEGC], AF.Identity,
                                         bias=obt[0:1, 0:1])
                    nc.sync.dma_start(out[None, :], ov[:])

    nc.compile()
    return nc


def prep_inputs(inputs, cfg: CFG):
    """Full (unsharded) numpy inputs -> list of 8 per-core input dicts."""
    c = cfg
    f32 = np.float32
    atom_fea = np.asarray(inputs["atom_fea"], f32)
    nbr_fea = np.asarray(inputs["nbr_fea"], f32)
    nbr_idx = np.asarray(inputs["nbr_fea_idx"]).astype(np.int64)
    M1 = np.asarray(inputs["M1_index"]).astype(np.int64)
    seg = np.asarray(inputs["seg_ids"]).astype(np.int64)
    m2 = np.asarray(inputs["m2_fea"], f32)
    w1 = np.asarray(inputs["w1"], f32)
    b1 = np.asarray(inputs["b1"], f32)
    w2 = np.asarray(inputs["w2"], f32)
    gam = np.asarray(inputs["gamma"], f32)
    bet = np.asarray(inputs["beta"], f32)

    # shared weight tensors
    shared = {}
    shared["emb_w"] = np.asarray(inputs["emb_w"], f32)
    shared["emb_b"] = np.asarray(inputs["emb_b"], f32).reshape(c.OT, 128).T.copy()
    shared["w1s"] = w1[:, : c.AFD].reshape(c.N_CONV, 2, 128, 2 * c.AFD).copy()
    shared["w1n"] = w1[:, c.AFD:2 * c.AFD].reshape(c.N_CONV, 2, 128, 2 * c.AFD).copy()
    shared["w1e"] = w1[:, 2 * c.AFD:].copy()
    shared["b1"] = np.concatenate(
        [b1[i].reshape(c.CT, 128).T for i in range(c.N_CONV)], 1)
    shared["w2"] = w2.reshape(c.N_CONV, c.CT, 128, c.AFD).copy()
    shared["gamma"] = np.concatenate(
        [gam[i].reshape(c.OT, 128).T for i in range(c.N_CONV)], 1)
    shared["beta"] = np.concatenate(
        [bet[i].reshape(c.OT, 128).T for i in range(c.N_CONV)], 1)
    fc_w = np.asarray(inputs["fc_w"], f32)
    shared["fc_w0"] = fc_w[0:128].copy()
    shared["fc_w1"] = fc_w[128:256].copy()
    shared["fc_w2"] = fc_w[256:260].copy()
    shared["fc_b"] = np.asarray(inputs["fc_b"], f32).reshape(c.H // 128, 128).T.copy()
    shared["out_w"] = np.asarray(inputs["out_w"], f32).reshape(c.H // 128, 128).T.copy()
    shared["out_b"] = np.asarray(inputs["out_b"], f32).reshape(1, 1)

    remap = (nbr_idx // c.NV) * c.NLP + (nbr_idx % c.NV)   # -> global padded rows
    m1re = (M1 // c.NV) * c.NLP + (M1 % c.NV)
    counts = np.bincount(seg, minlength=c.B).astype(f32)
    counts[counts == 0] = 1.0

    in_maps = []
    for cc in range(c.NC):
        d = dict(shared)
        s = slice(cc * c.NV, (cc + 1) * c.NV)
        atomT = np.zeros((c.ORIG, c.NLP), f32)
        atomT[:, : c.NV] = atom_fea[s].T
        d["atomT"] = atomT
        nbrT = np.zeros((c.M, c.NBR, c.NLP), f32)
        nbrT[:, :, : c.NV] = nbr_fea[s].transpose(1, 2, 0)
        d["nbrT"] = nbrT
        # gather indices, m-major, per 512-atom block
        gi = np.zeros((128, c.M * c.NAB * (c.ABLK // 16)), np.int16)
        rloc = remap[s]                                   # [NV, M]
        for m in range(c.M):
            col = np.zeros(c.NLP, np.int64)
            col[: c.NV] = rloc[:, m]
            for ab in range(c.NAB):
                w = wrap16(col[ab * c.ABLK:(ab + 1) * c.ABLK], c.ABLK)
                i0 = (m * c.NAB + ab) * (c.ABLK // 16)
                gi[:, i0: i0 + c.ABLK // 16] = w
        d["gidx"] = gi
        # pooling rows/segments for this core
        rs = slice(cc * c.BK, (cc + 1) * c.BK)
        d["m1idx"] = wrap16(m1re[rs], c.BKP)
        sel = np.zeros((128, c.NMB * c.SEGC), f32)
        segs = seg[rs] - cc * c.SEGC
        for r in range(c.BK):
            b, sgl = r // 128, segs[r]
            if 0 <= sgl < c.SEGC:
                sel[r % 128, b * c.SEGC + sgl] = 1.0 / counts[seg[rs][r]]
        d["sel"] = sel
        d["m2T"] = m2[cc * c.SEGC:(cc + 1) * c.SEGC].T.copy()
        in_maps.append(d)
    return in_maps


_CACHE = {}


def get_program(cfg=None):
    cfg = cfg or CFG()
    key = tuple(sorted(cfg.__dict__.items()))
    if key not in _CACHE:
        _CACHE[key] = build_program(cfg)
    return _CACHE[key]


def kernel(**inputs):
    cfg = CFG()
    nc = get_program(cfg)
    in_maps = prep_inputs(inputs, cfg)
    res = run_bass_kernel_spmd(nc, in_maps, core_ids=list(range(cfg.NC)))
    out = np.concatenate([r["o_out"] for r in res.results])
    return out.reshape(cfg.B, 1).astype(np.float32)

